# revision 15
# baseline (speedup 1.0000x reference)
"""Multi-level (FPN) DeformRoIPool (zero-offset == aligned RoIAlign) for Trainium2.

Strategy (8 NeuronCores, SPMD, one Bass program):
- The bin/sample grid spacing is always < 2 px, so the set of pixels a ROI
  needs is exactly the dense bounding box of its sample corners. Host crops
  that box per ROI (channels-last), quantizes to int8 (rel err ~9e-3, well
  under the 2e-2 gate), and packs all of a core's crops into one contiguous
  "stream" (row = one pixel = 256 ch).
- Bilinear + sample-average reduction is separable: out[49, C] = W^T @ crop
  with W = (Ay (x) Ax) * SF built per ROI on host (SF folds the int8 scale).
  Device does K=128 fp16 matmuls (pixels on the partition dim) accumulating
  in PSUM [49, 256].
- ROIs are snake-dealt to cores by crop size; per-slot stream offsets are
  padded to the max across cores so the matmul schedule (group -> slot,
  start/stop) is identical on every core: SPMD-uniform program, raggedness
  lives in the data (stream contents + per-set weight tiles).
- Stream is stored pre-swizzled [128, G*256] int8 and DMA'd in ramped chunks
  via the gpsimd SWDGE ring with int8->fp16 cast (halves HBM read bytes);
  weights ride the SP HWDGE ring, outputs the ACT ring: three independent
  DMA paths. PE is pre-warmed with dummy matmuls so HAM reaches full clock
  before the first real matmul.
"""
import numpy as np

OUT = 7
SR = 2
STRIDES = (4, 8, 16, 32)
FINEST = 56.0
NLEV = 4
C = 256
N_ROIS = 256
N_CORES = 8
NROI_C = N_ROIS // N_CORES          # 32 roi slots per core
CH_MAX = 13                         # steady-state groups per stream chunk
SF = np.float32(4.0 / 127.0)        # int8 feature scale (clip at 4 sigma)
N_WARM = 48                         # PE warmup matmuls
OB = 4                              # roi slots per output DMA batch
FEAT_SHAPES = [(2, 256, 200, 200), (2, 256, 100, 100), (2, 256, 50, 50), (2, 256, 25, 25)]


# ---------------------------------------------------------------------------
# BIR fix: this container's walrus rejects >1 embedded sem wait per
# instruction (2 on EventSemaphore). Split excess waits onto EventSemaphore
# carriers at serialization time.
# ---------------------------------------------------------------------------
def _install_bir_waitsplit():
    import orjson
    import concourse.bass as bass

    if getattr(bass.Bass, "_waitsplit_patched", False):
        return

    def _fix_blocks(blocks, counter):
        for blk in blocks:
            insts = blk.get("instructions")
            if insts:
                out = []
                for ins in insts:
                    si = ins.get("sync_info")
                    ow = (si or {}).get("on_wait") or []
                    limit = 2 if ins.get("opcode") == "EventSemaphore" else 1
                    if len(ow) > limit:
                        excess = ow[: len(ow) - limit]
                        si["on_wait"] = ow[len(ow) - limit:]
                        for i in range(0, len(excess), 2):
                            counter[0] += 1
                            out.append({
                                "name": f"I-waitsplit-{counter[0]}",
                                "opcode": "EventSemaphore",
                                "engine": ins["engine"],
                                "ins": [], "outs": [],
                                "debug": ins.get("debug", 0),
                                "sync_info": {"on_update": [], "on_wait": excess[i:i + 2]},
                            })
                    out.append(ins)
                blk["instructions"] = out
            if blk.get("blocks"):
                _fix_blocks(blk["blocks"], counter)

    orig = bass.Bass.to_json_bytes

    def to_json_bytes(self, *a, **kw):
        data = orig(self, *a, **kw)
        d = orjson.loads(data)
        counter = [0]
        for fn in d.get("functions", []):
            _fix_blocks(fn.get("blocks", []), counter)
        return orjson.dumps(d) if counter[0] else data

    bass.Bass.to_json_bytes = to_json_bytes
    bass.Bass._waitsplit_patched = True


# ---------------------------------------------------------------------------
# Host-side crop / weight computation
# ---------------------------------------------------------------------------
def _roi_meta(rois):
    """Per-roi level, crop bbox, and separable row/col weight matrices."""
    scale = np.sqrt((rois[:, 3] - rois[:, 1]) * (rois[:, 4] - rois[:, 2]))  # f32, as jax
    tl_f = np.clip(np.floor(np.log2(scale / np.float32(FINEST) + np.float32(1e-6))), 0, NLEV - 1)
    tl = (tl_f + 1e-5).astype(np.int32)
    g = np.arange(OUT, dtype=np.float64)[:, None] + (np.arange(SR, dtype=np.float64)[None, :] + 0.5) / SR
    metas = []
    for n in range(rois.shape[0]):
        l = int(tl[n])
        _, _, H, W = FEAT_SHAPES[l]
        sc = 1.0 / STRIDES[l]
        x1 = rois[n, 1] * sc - 0.5
        y1 = rois[n, 2] * sc - 0.5
        rw = rois[n, 3] * sc - 0.5 - x1
        rh = rois[n, 4] * sc - 0.5 - y1
        y = y1 + (rh / OUT) * g   # [OUT, SR]
        x = x1 + (rw / OUT) * g
        vy = (y > -1) & (y < H)
        vx = (x > -1) & (x < W)
        yc = np.clip(y, 0.0, H - 1)
        xc = np.clip(x, 0.0, W - 1)
        y0 = np.minimum(np.floor(yc).astype(np.int64), H - 1)
        x0 = np.minimum(np.floor(xc).astype(np.int64), W - 1)
        y1i = np.minimum(y0 + 1, H - 1)
        x1i = np.minimum(x0 + 1, W - 1)
        ly = yc - y0
        lx = xc - x0
        ymin, ymax = int(y0.min()), int(y1i.max())
        xmin, xmax = int(x0.min()), int(x1i.max())
        R, S = ymax - ymin + 1, xmax - xmin + 1
        Ay = np.zeros((R, OUT))
        Ax = np.zeros((S, OUT))
        for i in range(OUT):
            for si in range(SR):
                v = vy[i, si] * 0.5
                Ay[y0[i, si] - ymin, i] += (1.0 - ly[i, si]) * v
                Ay[y1i[i, si] - ymin, i] += ly[i, si] * v
                v = vx[i, si] * 0.5
                Ax[x0[i, si] - xmin, i] += (1.0 - lx[i, si]) * v
                Ax[x1i[i, si] - xmin, i] += lx[i, si] * v
        metas.append(dict(l=l, b=int(rois[n, 0]), ymin=ymin, xmin=xmin, R=R, S=S,
                          Ay=Ay, Ax=Ax, rows=R * S))
    return metas


def _plan(metas):
    """Snake-deal rois to cores by crop size; common per-slot row boundaries."""
    sizes = np.array([m["rows"] for m in metas])
    order = np.argsort(-sizes, kind="stable")
    cores = [[] for _ in range(N_CORES)]
    for k, n in enumerate(order):
        r, j = divmod(k, N_CORES)
        c = j if r % 2 == 0 else N_CORES - 1 - j
        cores[c].append(int(n))
    percore = np.array([[sizes[n] for n in cl] for cl in cores])       # [8, 32]
    bounds = np.cumsum(percore.max(axis=0)).astype(np.int64)           # common B_k
    total = int(bounds[-1])
    G = -(-total // 128)
    # ramped chunk sizes: small first chunks for a fast pipeline start
    chs = []
    for w in (2, 4, 8):
        if sum(chs) + w <= G:
            chs.append(w)
    while G - sum(chs) > CH_MAX:
        chs.append(CH_MAX)
    if G - sum(chs) > 0:
        chs.append(G - sum(chs))
    chb = np.concatenate([[0], np.cumsum(chs)]).astype(np.int64)       # chunk bounds
    # uniform set list: (slot, group, start, stop)
    sets = []
    for k in range(NROI_C):
        lo = 0 if k == 0 else int(bounds[k - 1])
        hi = int(bounds[k])
        g0, g1 = lo // 128, (hi - 1) // 128
        for gi in range(g0, g1 + 1):
            sets.append((k, gi, gi == g0, gi == g1))
    return cores, bounds, G, chb, sets


def _build_core_inputs(feats_T, metas, core_rois, bounds, G, sets):
    nsets = len(sets)
    stream = np.zeros((G * 128, C), np.int8)
    wts = np.zeros((nsets, 128, 49), np.float16)
    set_idx = {}
    for s, (k, gi, _, _) in enumerate(sets):
        set_idx[(k, gi)] = s
    inv_sf = 1.0 / float(SF)
    for k, n in enumerate(core_rois):
        m = metas[n]
        lo = 0 if k == 0 else int(bounds[k - 1])
        fT = feats_T[m["l"]][m["b"]]
        crop = fT[m["ymin"]:m["ymin"] + m["R"], m["xmin"]:m["xmin"] + m["S"], :]
        q = np.clip(np.rint(crop.reshape(m["rows"], C) * inv_sf), -127, 127)
        stream[lo:lo + m["rows"]] = q.astype(np.int8)
        Wf = (m["Ay"][:, None, :, None] * m["Ax"][None, :, None, :]).reshape(m["rows"], 49)
        Wf = Wf * float(SF)
        r = 0
        while r < m["rows"]:
            gr = lo + r
            gi = gr // 128
            p = gr - gi * 128
            take = min(128 - p, m["rows"] - r)
            wts[set_idx[(k, gi)], p:p + take] = Wf[r:r + take]
            r += take
    # pre-swizzle: stream row (g*128+p) -> [p, g*256 + c]
    stream_sw = np.ascontiguousarray(
        stream.reshape(G, 128, C).transpose(1, 0, 2)).reshape(128, G * C)
    wts_sw = np.ascontiguousarray(wts.transpose(1, 0, 2)).reshape(128, nsets * 49)
    return stream_sw, wts_sw


# ---------------------------------------------------------------------------
# Device program
# ---------------------------------------------------------------------------
def _build_program(G, chb, sets):
    import concourse.bacc as bacc
    import concourse.mybir as mybir
    import concourse.tile as tile

    _install_bir_waitsplit()
    nc = bacc.Bacc("TRN2", debug=False, enable_asserts=True, num_devices=N_CORES)

    nsets = len(sets)
    nch = len(chb) - 1
    chunk_slo = []
    for c in range(nch):
        chunk_slo.append(sum(1 for (_, gi, _, _) in sets if gi < chb[c]))
    chunk_slo.append(nsets)
    ns_max = max(chunk_slo[c + 1] - chunk_slo[c] for c in range(nch))
    ch_max = max(int(chb[c + 1] - chb[c]) for c in range(nch))

    stream_d = nc.dram_tensor("stream", [128, G * C], mybir.dt.int8, kind="ExternalInput")
    wts_d = nc.dram_tensor("wts", [128, nsets * 49], mybir.dt.float16, kind="ExternalInput")
    out_d = nc.dram_tensor("out", [NROI_C, 49 * C], mybir.dt.float16, kind="ExternalOutput")

    with tile.TileContext(nc) as tc:
        with (
            tc.tile_pool(name="ip", bufs=1) as ip,
            tc.tile_pool(name="gp", bufs=4) as gp,
            tc.tile_pool(name="wp", bufs=4) as wp,
            tc.tile_pool(name="sp", bufs=3) as sp,
            tc.tile_pool(name="pp", bufs=7, space="PSUM") as pp,
            tc.tile_pool(name="ppw", bufs=1, space="PSUM") as ppw,
        ):
            # PE warmup: get HAM to full clock before the first real matmul
            zl = ip.tile([128, 49], mybir.dt.float16)
            zr = ip.tile([128, C], mybir.dt.float16)
            nc.vector.memset(zl[:], 0.0)
            nc.vector.memset(zr[:], 0.0)
            ps_w = ppw.tile([49, C], mybir.dt.float32, tag="warm", name="ps_warm")
            for i in range(N_WARM):
                nc.tensor.matmul(out=ps_w[:, :], lhsT=zl[:], rhs=zr[:],
                                 start=(i == 0), stop=(i == N_WARM - 1))

            gt = {}
            wt = {}

            def emit_chunk(c):
                glo, ghi = int(chb[c]), int(chb[c + 1])
                g = gp.tile([128, ch_max * C], mybir.dt.float16, tag="g", name=f"g_{c}")
                # SWDGE ring: int8 -> fp16 cast during DMA (halves HBM reads)
                nc.gpsimd.dma_start(g[:, 0:(ghi - glo) * C], stream_d[:, glo * C:ghi * C])
                w = wp.tile([128, ns_max * 49], mybir.dt.float16, tag="w", name=f"w_{c}")
                s0, s1 = chunk_slo[c], chunk_slo[c + 1]
                nc.sync.dma_start(w[:, 0:(s1 - s0) * 49], wts_d[:, s0 * 49:s1 * 49])
                gt[c] = g
                wt[c] = w

            g2c = {}
            for c in range(nch):
                for gi in range(int(chb[c]), int(chb[c + 1])):
                    g2c[gi] = c

            emitted = -1
            ps = None
            st = None
            for s, (k, gi, first, last) in enumerate(sets):
                c = g2c[gi]
                while emitted < c:
                    emitted += 1
                    emit_chunk(emitted)
                if first:
                    ps = pp.tile([49, C], mybir.dt.float32, tag="ps", name=f"ps_{k}")
                if k % OB == 0 and first:
                    st = sp.tile([49, OB * C], mybir.dt.float16, tag="st", name=f"st_{k // OB}")
                nc.tensor.matmul(
                    out=ps[:, :],
                    lhsT=wt[c][:, (s - chunk_slo[c]) * 49:(s - chunk_slo[c] + 1) * 49],
                    rhs=gt[c][:, (gi - int(chb[c])) * C:(gi - int(chb[c]) + 1) * C],
                    start=first,
                    stop=last,
                )
                if last:
                    # alternate PSUM->SBUF copies across DVE and ACT
                    if k % 2 == 0:
                        nc.vector.tensor_copy(st[:, (k % OB) * C:(k % OB + 1) * C], ps[:])
                    else:
                        nc.scalar.copy(st[:, (k % OB) * C:(k % OB + 1) * C], ps[:])
                    if k % OB == OB - 1:
                        eng = nc.sync if (k // OB) % 2 == 0 else nc.scalar
                        eng.dma_start(
                            out_d[k - (OB - 1):k + 1].rearrange("r (b c) -> b r c", c=C),
                            st[:].rearrange("b (r c) -> b r c", c=C),
                        )
    nc.compile()
    return nc


def kernel(feat0, feat1, feat2, feat3, rois):
    from concourse.bass_utils import run_bass_kernel_spmd

    feats = [np.asarray(f, np.float32) for f in (feat0, feat1, feat2, feat3)]
    rois = np.asarray(rois, np.float32)
    feats_T = [np.ascontiguousarray(f.transpose(0, 2, 3, 1)) for f in feats]
    metas = _roi_meta(rois)
    cores, bounds, G, chb, sets = _plan(metas)

    in_maps = []
    for core in range(N_CORES):
        stream_sw, wts_sw = _build_core_inputs(feats_T, metas, cores[core], bounds, G, sets)
        in_maps.append({"stream": stream_sw, "wts": wts_sw})

    nc = _build_program(G, chb, sets)
    res = run_bass_kernel_spmd(nc, in_maps, core_ids=list(range(N_CORES)), trace=False)
    out = np.zeros((N_ROIS, C, OUT, OUT), np.float32)
    for core in range(N_CORES):
        o = res.results[core]["out"].astype(np.float32).reshape(NROI_C, 49, C)
        o = o.transpose(0, 2, 1).reshape(NROI_C, C, OUT, OUT)
        for k, n in enumerate(cores[core]):
            out[n] = o[k]
    return out


# Testing hook: emulate the device math in numpy (same stream/weight data).
def emulate(feat0, feat1, feat2, feat3, rois):
    feats = [np.asarray(f, np.float32) for f in (feat0, feat1, feat2, feat3)]
    rois = np.asarray(rois, np.float32)
    feats_T = [np.ascontiguousarray(f.transpose(0, 2, 3, 1)) for f in feats]
    metas = _roi_meta(rois)
    cores, bounds, G, chb, sets = _plan(metas)
    out = np.zeros((N_ROIS, C, OUT, OUT), np.float32)
    for core in range(N_CORES):
        stream_sw, wts_sw = _build_core_inputs(feats_T, metas, cores[core], bounds, G, sets)
        stream = stream_sw.reshape(128, G, C).transpose(1, 0, 2).astype(np.float32)
        wts = wts_sw.reshape(128, len(sets), 49).transpose(1, 0, 2).astype(np.float32)
        accs = {}
        for s, (k, gi, first, last) in enumerate(sets):
            if first:
                accs[k] = np.zeros((49, C), np.float32)
            accs[k] += wts[s].T @ stream[gi]
            if last:
                n = cores[core][k]
                out[n] = accs[k].T.reshape(C, OUT, OUT)
    return out


# revision 16
# speedup vs baseline: 1.0134x; 1.0134x over previous
"""Multi-level (FPN) DeformRoIPool (zero-offset == aligned RoIAlign) for Trainium2.

Strategy (8 NeuronCores, SPMD, one Bass program):
- The bin/sample grid spacing is always < 2 px, so the set of pixels a ROI
  needs is exactly the dense bounding box of its sample corners. Host crops
  that box per ROI (channels-last fp16) and packs all of a core's crops into
  one contiguous "stream" (row = one pixel = 256 ch).
- Bilinear + sample-average reduction is separable: out[49, C] = W^T @ crop
  with W = Ay (x) Ax built per ROI on host. Device does K=128 fp16 matmuls
  (pixels on the partition dim) accumulating in PSUM [49, 256].
- ROIs are snake-dealt to cores by crop size; per-slot stream offsets are
  padded to the max across cores so the matmul schedule (group -> slot,
  start/stop) is identical on every core: SPMD-uniform program, raggedness
  lives in the data (stream contents + per-set weight tiles).
- DMA descriptor efficiency: stream groups AND the chunk's weight tiles are
  fused into one DRAM region per chunk, contiguous per partition -> one
  dma_start per chunk with ~16 KB descriptors. Output DRAM is laid out
  [49 bins, roi*C] so the two output DMAs use 8 KB descriptors. Chunk DMAs
  alternate the two HWDGE rings (SP/ACT). Chunk sizes ramp up (fast
  pipeline start) and down (short tail); every chunk owns its buffer.
- PE is pre-warmed with dummy matmuls so HAM is at full clock for the real
  work.
"""
import numpy as np

OUT = 7
SR = 2
STRIDES = (4, 8, 16, 32)
FINEST = 56.0
NLEV = 4
C = 256
N_ROIS = 256
N_CORES = 8
NROI_C = N_ROIS // N_CORES          # 32 roi slots per core
N_WARM = 48                         # PE warmup matmuls
OB = 16                             # roi slots per output DMA batch
FEAT_SHAPES = [(2, 256, 200, 200), (2, 256, 100, 100), (2, 256, 50, 50), (2, 256, 25, 25)]


# ---------------------------------------------------------------------------
# BIR fix: this container's walrus rejects >1 embedded sem wait per
# instruction (2 on EventSemaphore). Split excess waits onto EventSemaphore
# carriers at serialization time.
# ---------------------------------------------------------------------------
def _install_bir_waitsplit():
    import orjson
    import concourse.bass as bass

    if getattr(bass.Bass, "_waitsplit_patched", False):
        return

    def _fix_blocks(blocks, counter):
        for blk in blocks:
            insts = blk.get("instructions")
            if insts:
                out = []
                for ins in insts:
                    si = ins.get("sync_info")
                    ow = (si or {}).get("on_wait") or []
                    limit = 2 if ins.get("opcode") == "EventSemaphore" else 1
                    if len(ow) > limit:
                        excess = ow[: len(ow) - limit]
                        si["on_wait"] = ow[len(ow) - limit:]
                        for i in range(0, len(excess), 2):
                            counter[0] += 1
                            out.append({
                                "name": f"I-waitsplit-{counter[0]}",
                                "opcode": "EventSemaphore",
                                "engine": ins["engine"],
                                "ins": [], "outs": [],
                                "debug": ins.get("debug", 0),
                                "sync_info": {"on_update": [], "on_wait": excess[i:i + 2]},
                            })
                    out.append(ins)
                blk["instructions"] = out
            if blk.get("blocks"):
                _fix_blocks(blk["blocks"], counter)

    orig = bass.Bass.to_json_bytes

    def to_json_bytes(self, *a, **kw):
        data = orig(self, *a, **kw)
        d = orjson.loads(data)
        counter = [0]
        for fn in d.get("functions", []):
            _fix_blocks(fn.get("blocks", []), counter)
        return orjson.dumps(d) if counter[0] else data

    bass.Bass.to_json_bytes = to_json_bytes
    bass.Bass._waitsplit_patched = True


# ---------------------------------------------------------------------------
# Host-side crop / weight computation
# ---------------------------------------------------------------------------
def _roi_meta(rois):
    """Per-roi level, crop bbox, and separable row/col weight matrices."""
    scale = np.sqrt((rois[:, 3] - rois[:, 1]) * (rois[:, 4] - rois[:, 2]))  # f32, as jax
    tl_f = np.clip(np.floor(np.log2(scale / np.float32(FINEST) + np.float32(1e-6))), 0, NLEV - 1)
    tl = (tl_f + 1e-5).astype(np.int32)
    g = np.arange(OUT, dtype=np.float64)[:, None] + (np.arange(SR, dtype=np.float64)[None, :] + 0.5) / SR
    metas = []
    for n in range(rois.shape[0]):
        l = int(tl[n])
        _, _, H, W = FEAT_SHAPES[l]
        sc = 1.0 / STRIDES[l]
        x1 = rois[n, 1] * sc - 0.5
        y1 = rois[n, 2] * sc - 0.5
        rw = rois[n, 3] * sc - 0.5 - x1
        rh = rois[n, 4] * sc - 0.5 - y1
        y = y1 + (rh / OUT) * g   # [OUT, SR]
        x = x1 + (rw / OUT) * g
        vy = (y > -1) & (y < H)
        vx = (x > -1) & (x < W)
        yc = np.clip(y, 0.0, H - 1)
        xc = np.clip(x, 0.0, W - 1)
        y0 = np.minimum(np.floor(yc).astype(np.int64), H - 1)
        x0 = np.minimum(np.floor(xc).astype(np.int64), W - 1)
        y1i = np.minimum(y0 + 1, H - 1)
        x1i = np.minimum(x0 + 1, W - 1)
        ly = yc - y0
        lx = xc - x0
        ymin, ymax = int(y0.min()), int(y1i.max())
        xmin, xmax = int(x0.min()), int(x1i.max())
        R, S = ymax - ymin + 1, xmax - xmin + 1
        Ay = np.zeros((R, OUT))
        Ax = np.zeros((S, OUT))
        for i in range(OUT):
            for si in range(SR):
                v = vy[i, si] * 0.5
                Ay[y0[i, si] - ymin, i] += (1.0 - ly[i, si]) * v
                Ay[y1i[i, si] - ymin, i] += ly[i, si] * v
                v = vx[i, si] * 0.5
                Ax[x0[i, si] - xmin, i] += (1.0 - lx[i, si]) * v
                Ax[x1i[i, si] - xmin, i] += lx[i, si] * v
        metas.append(dict(l=l, b=int(rois[n, 0]), ymin=ymin, xmin=xmin, R=R, S=S,
                          Ay=Ay, Ax=Ax, rows=R * S))
    return metas


def _chunk_sizes(G):
    """Ramped chunk sizes: up for a fast pipeline start, down for a short tail."""
    chs = []
    for w in (2, 4, 8, 13):
        if sum(chs) + w <= G:
            chs.append(w)
    while G - sum(chs) >= 33:
        chs.append(25)
    rem = G - sum(chs)
    if rem > 12:
        chs.extend([rem - 8, 8])
    elif rem > 0:
        chs.append(rem)
    return chs


def _plan(metas):
    """Snake-deal rois to cores by crop size; common per-slot row boundaries."""
    sizes = np.array([m["rows"] for m in metas])
    order = np.argsort(-sizes, kind="stable")
    cores = [[] for _ in range(N_CORES)]
    for k, n in enumerate(order):
        r, j = divmod(k, N_CORES)
        c = j if r % 2 == 0 else N_CORES - 1 - j
        cores[c].append(int(n))
    percore = np.array([[sizes[n] for n in cl] for cl in cores])       # [8, 32]
    bounds = np.cumsum(percore.max(axis=0)).astype(np.int64)           # common B_k
    total = int(bounds[-1])
    G = -(-total // 128)
    chb = np.concatenate([[0], np.cumsum(_chunk_sizes(G))]).astype(np.int64)
    # uniform set list: (slot, group, start, stop)
    sets = []
    for k in range(NROI_C):
        lo = 0 if k == 0 else int(bounds[k - 1])
        hi = int(bounds[k])
        g0, g1 = lo // 128, (hi - 1) // 128
        for gi in range(g0, g1 + 1):
            sets.append((k, gi, gi == g0, gi == g1))
    # chunk layout: per chunk [CH*C stream cols | ns_c*49 weight cols]
    nch = len(chb) - 1
    chunk_slo = [sum(1 for (_, gi, _, _) in sets if gi < chb[c]) for c in range(nch)]
    chunk_slo.append(len(sets))
    col_off = [0]
    for c in range(nch):
        ch_c = int(chb[c + 1] - chb[c])
        ns_c = chunk_slo[c + 1] - chunk_slo[c]
        col_off.append(col_off[-1] + ch_c * C + ns_c * 49)
    return cores, bounds, G, chb, sets, chunk_slo, col_off


def _build_core_inputs(feats_T, metas, core_rois, bounds, G, sets, chb, chunk_slo, col_off):
    nsets = len(sets)
    stream = np.zeros((G * 128, C), np.float16)
    wts = np.zeros((nsets, 128, 49), np.float16)
    set_idx = {}
    for s, (k, gi, _, _) in enumerate(sets):
        set_idx[(k, gi)] = s
    for k, n in enumerate(core_rois):
        m = metas[n]
        lo = 0 if k == 0 else int(bounds[k - 1])
        fT = feats_T[m["l"]][m["b"]]
        crop = fT[m["ymin"]:m["ymin"] + m["R"], m["xmin"]:m["xmin"] + m["S"], :]
        stream[lo:lo + m["rows"]] = crop.reshape(m["rows"], C)
        Wf = (m["Ay"][:, None, :, None] * m["Ax"][None, :, None, :]).reshape(m["rows"], 49)
        r = 0
        while r < m["rows"]:
            gr = lo + r
            gi = gr // 128
            p = gr - gi * 128
            take = min(128 - p, m["rows"] - r)
            wts[set_idx[(k, gi)], p:p + take] = Wf[r:r + take]
            r += take
    # fused per-chunk layout: [p, stream cols of chunk | weight cols of chunk]
    sg = stream.reshape(G, 128, C)
    nch = len(chb) - 1
    data = np.empty((128, col_off[-1]), np.float16)
    for c in range(nch):
        glo, ghi = int(chb[c]), int(chb[c + 1])
        s0, s1 = chunk_slo[c], chunk_slo[c + 1]
        off = col_off[c]
        scols = (ghi - glo) * C
        data[:, off:off + scols] = sg[glo:ghi].transpose(1, 0, 2).reshape(128, scols)
        wcols = (s1 - s0) * 49
        data[:, off + scols:off + scols + wcols] = (
            wts[s0:s1].transpose(1, 0, 2).reshape(128, wcols))
    return data


# ---------------------------------------------------------------------------
# Device program
# ---------------------------------------------------------------------------
def _build_program(G, chb, sets, chunk_slo, col_off):
    import concourse.bacc as bacc
    import concourse.mybir as mybir
    import concourse.tile as tile

    _install_bir_waitsplit()
    nc = bacc.Bacc("TRN2", debug=False, enable_asserts=True, num_devices=N_CORES)

    nch = len(chb) - 1

    data_d = nc.dram_tensor("data", [128, col_off[-1]], mybir.dt.float16, kind="ExternalInput")
    out_d = nc.dram_tensor("out", [49, NROI_C * C], mybir.dt.float16, kind="ExternalOutput")

    with tile.TileContext(nc) as tc:
        with (
            tc.tile_pool(name="ip", bufs=1) as ip,
            tc.tile_pool(name="gp", bufs=1) as gp,
            tc.tile_pool(name="sp", bufs=2) as sp,
            tc.tile_pool(name="pp", bufs=7, space="PSUM") as pp,
            tc.tile_pool(name="ppw", bufs=1, space="PSUM") as ppw,
        ):
            # PE warmup: get HAM to full clock before the first real matmul
            zl = ip.tile([128, 49], mybir.dt.float16)
            zr = ip.tile([128, C], mybir.dt.float16)
            nc.vector.memset(zl[:], 0.0)
            nc.vector.memset(zr[:], 0.0)
            ps_w = ppw.tile([49, C], mybir.dt.float32, tag="warm", name="ps_warm")
            for i in range(N_WARM):
                nc.tensor.matmul(out=ps_w[:, :], lhsT=zl[:], rhs=zr[:],
                                 start=(i == 0), stop=(i == N_WARM - 1))

            ct = {}

            def emit_chunk(c):
                ncols = col_off[c + 1] - col_off[c]
                t = gp.tile([128, ncols], mybir.dt.float16, tag=f"ck{c}", name=f"ck_{c}")
                eng = nc.sync if c % 2 == 0 else nc.scalar
                eng.dma_start(t[:], data_d[:, col_off[c]:col_off[c + 1]])
                ct[c] = t

            g2c = {}
            for c in range(nch):
                for gi in range(int(chb[c]), int(chb[c + 1])):
                    g2c[gi] = c

            emitted = -1
            ps = None
            st = None
            for s, (k, gi, first, last) in enumerate(sets):
                c = g2c[gi]
                while emitted < c:
                    emitted += 1
                    emit_chunk(emitted)
                if first:
                    ps = pp.tile([49, C], mybir.dt.float32, tag="ps", name=f"ps_{k}")
                if k % OB == 0 and first:
                    st = sp.tile([49, OB * C], mybir.dt.float16, tag="st", name=f"st_{k // OB}")
                t = ct[c]
                wbase = (int(chb[c + 1]) - int(chb[c])) * C
                nc.tensor.matmul(
                    out=ps[:, :],
                    lhsT=t[:, wbase + (s - chunk_slo[c]) * 49:wbase + (s - chunk_slo[c] + 1) * 49],
                    rhs=t[:, (gi - int(chb[c])) * C:(gi - int(chb[c]) + 1) * C],
                    start=first,
                    stop=last,
                )
                if last:
                    # alternate PSUM->SBUF copies across DVE (fast) and ACT
                    if k % 3 == 0:
                        nc.scalar.copy(st[:, (k % OB) * C:(k % OB + 1) * C], ps[:])
                    else:
                        nc.vector.tensor_copy(st[:, (k % OB) * C:(k % OB + 1) * C], ps[:])
                    if k % OB == OB - 1:
                        eng = nc.scalar if (k // OB) % 2 == 0 else nc.sync
                        eng.dma_start(
                            out_d[:, (k - (OB - 1)) * C:(k + 1) * C],
                            st[:],
                        )
    nc.compile()
    return nc


def kernel(feat0, feat1, feat2, feat3, rois):
    from concourse.bass_utils import run_bass_kernel_spmd

    feats = [np.asarray(f, np.float32) for f in (feat0, feat1, feat2, feat3)]
    rois = np.asarray(rois, np.float32)
    feats_T = [np.ascontiguousarray(f.transpose(0, 2, 3, 1)) for f in feats]
    metas = _roi_meta(rois)
    cores, bounds, G, chb, sets, chunk_slo, col_off = _plan(metas)

    in_maps = []
    for core in range(N_CORES):
        data = _build_core_inputs(feats_T, metas, cores[core], bounds, G, sets,
                                  chb, chunk_slo, col_off)
        in_maps.append({"data": data})

    nc = _build_program(G, chb, sets, chunk_slo, col_off)
    res = run_bass_kernel_spmd(nc, in_maps, core_ids=list(range(N_CORES)), trace=False)
    out = np.zeros((N_ROIS, C, OUT, OUT), np.float32)
    for core in range(N_CORES):
        o = res.results[core]["out"].astype(np.float32).reshape(49, NROI_C, C)
        o = o.transpose(1, 2, 0).reshape(NROI_C, C, OUT, OUT)
        for k, n in enumerate(cores[core]):
            out[n] = o[k]
    return out


# Testing hook: emulate the device math in numpy (same packed data).
def emulate(feat0, feat1, feat2, feat3, rois):
    feats = [np.asarray(f, np.float32) for f in (feat0, feat1, feat2, feat3)]
    rois = np.asarray(rois, np.float32)
    feats_T = [np.ascontiguousarray(f.transpose(0, 2, 3, 1)) for f in feats]
    metas = _roi_meta(rois)
    cores, bounds, G, chb, sets, chunk_slo, col_off = _plan(metas)
    g2c = {}
    for c in range(len(chb) - 1):
        for gi in range(int(chb[c]), int(chb[c + 1])):
            g2c[gi] = c
    out = np.zeros((N_ROIS, C, OUT, OUT), np.float32)
    for core in range(N_CORES):
        data = _build_core_inputs(feats_T, metas, cores[core], bounds, G, sets,
                                  chb, chunk_slo, col_off).astype(np.float32)
        accs = {}
        for s, (k, gi, first, last) in enumerate(sets):
            c = g2c[gi]
            off = col_off[c]
            wbase = off + (int(chb[c + 1]) - int(chb[c])) * C
            rhs = data[:, off + (gi - int(chb[c])) * C: off + (gi - int(chb[c]) + 1) * C]
            lhsT = data[:, wbase + (s - chunk_slo[c]) * 49: wbase + (s - chunk_slo[c] + 1) * 49]
            if first:
                accs[k] = np.zeros((49, C), np.float32)
            accs[k] += lhsT.T @ rhs
            if last:
                out[cores[core][k]] = accs[k].T.reshape(C, OUT, OUT)
    return out
